# revision 5
# baseline (speedup 1.0000x reference)
"""Trainium2 Bass kernel for nn_DLI_loss_full.

Key algebraic fact: logits[b,j,k] = hw[b,j] + xw[b,k] and the loss is
sum(lse - tgt) over valid groups, so the hw[b,j] term (the whole LSTM
path) cancels exactly:

    per_group[b,j] = log(sum_{k=j+1}^{len_b-1} exp(xw[b,k])) - xw[b,j+1]
    loss = sum(per_group) / sum_b(len_b - 1)

with xw = encoder_output @ w_fc[HID:].  The kernel therefore only
streams encoder_output once (memory-bound), computes xw via
multiply+256-wide reductions, then gets every suffix log-sum-exp with
one hardware suffix-sum scan per 48-wide chunk plus a cross-chunk
combine done as a tiny 128x128 matmul.

Per-core layout: 16 batches x 8 chunks of 48 timesteps = 128 SBUF
partitions, each partition's encoder slice contiguous in DRAM.  All
encoder DMAs ride the scalar HWDGE queue (a single queue sustains ~287+
B/ns under engine load; splitting the big stream across rings makes
every piece finish later due to packet round-robin).

v2 changes vs v1 (65.7us):
- consts ride the sync HWDGE ring (v1 had them on the gpsimd SWDGE
  path, landing at 13.6us and gating all compute; sync lands ~5us).
- gpsimd issues no DMAs -> no 16us dge_drain.
- free-axis grouped reduces are DVE-only on this ISA, so DVE does all
  8 reduces + 2 multiplies; gpsimd takes 6 multiplies (its 2-input TT
  is port-bound at ~2.6cyc/elem = ~3.5us/piece, so it is given the
  early pieces while DVE catches up on reduces).  DVE ops are ordered
  by expected data-readiness since each engine queue executes in-order.
- both Exp and Ln activation tables warm up front (two table slots, no
  eviction), removing a 1.5us ACT_TABLE_LOAD from the serial tail.
"""

from contextlib import ExitStack

import numpy as np

import concourse.bacc as bacc
import concourse.mybir as mybir
import concourse.tile as tile
from concourse import bass_utils

B, T, D, HID = 128, 384, 256, 256
NCORES = 8
BS = B // NCORES            # 16 batches per core
CH = 8                      # chunks per sequence
L = T // CH                 # 48 timesteps per chunk
P = BS * CH                 # 128 partitions
NP = 8                      # DMA/compute pieces along the free axis
LP = L // NP                # 6 timesteps per piece
F32 = mybir.dt.float32
I32 = mybir.dt.int32
EPS = 1e-30                 # keeps ln() finite on fully-masked tails

# pieces whose multiply runs on gpsimd; the rest multiply on DVE
GP_PIECES = (0, 1, 2, 3, 4)   # 5 early pieces; DVE mults 5-7; ACT reduces 2-4

_cache = {}


def _build_nc():
    nc = bacc.Bacc(
        "TRN2", target_bir_lowering=False, debug=False, num_devices=NCORES
    )
    x = nc.dram_tensor("x", [BS, T, D], F32, kind="ExternalInput").ap()
    mk = nc.dram_tensor("mk", [BS, T], I32, kind="ExternalInput").ap()
    wt = nc.dram_tensor("wt", [P, D], F32, kind="ExternalInput").ap()
    um = nc.dram_tensor("um", [P, P], F32, kind="ExternalInput").ap()
    cm = nc.dram_tensor("cm", [P, L], F32, kind="ExternalInput").ap()
    out = nc.dram_tensor("out", [P, 2], F32, kind="ExternalOutput").ap()

    add = mybir.AluOpType.add
    mult = mybir.AluOpType.mult
    bypass = mybir.AluOpType.bypass
    AX = mybir.AxisListType.X
    ACT = mybir.ActivationFunctionType

    with tile.TileContext(nc) as tc, ExitStack() as ctx:
        sp = ctx.enter_context(tc.tile_pool(name="small", bufs=1))
        xp = ctx.enter_context(tc.tile_pool(name="xp", bufs=NP))
        rp = ctx.enter_context(tc.tile_pool(name="prod", bufs=4))
        pp = ctx.enter_context(tc.tile_pool(name="psum", bufs=2, space="PSUM"))

        # x-piece loads first, all on the scalar HWDGE queue
        x_p = x.rearrange("b (c n l) d -> (b c) n (l d)", c=CH, n=NP)
        xts = []
        for i in range(NP):
            xt = xp.tile([P, LP * D], F32, tag="x")
            nc.scalar.dma_start(xt[:], x_p[:, i, :])
            xts.append(xt)

        # small constants ride the sync HWDGE ring - they land in a few
        # us without delaying the big stream's issue slots
        w_sb = sp.tile([P, D], F32)
        nc.sync.dma_start(w_sb[:], wt)
        cm_sb = sp.tile([P, L], F32)
        nc.sync.dma_start(cm_sb[:], cm)
        u_sb = sp.tile([P, P], F32)
        nc.sync.dma_start(u_sb[:], um)
        mi = sp.tile([P, L], I32)
        nc.sync.dma_start(mi[:], mk.rearrange("b (c l) -> (b c) l", c=CH))

        # warm BOTH activation tables while DMA streams (two table
        # slots, so Ln does not evict Exp); keeps the 1.5us
        # ACT_TABLE_LOAD for Ln off the serial tail
        warm = sp.tile([P, 1], F32)
        nc.scalar.activation(warm[:], cm_sb[:, 1:2], ACT.Exp)

        # replicate w LP times on-chip so the multiplies read a plain
        # contiguous operand (0-stride broadcast halves DVE rate)
        wrep = sp.tile([P, LP * D], F32)
        nc.vector.tensor_copy(wrep[:, 0:D], w_sb[:])
        nc.vector.tensor_copy(wrep[:, D:2 * D], wrep[:, 0:D])
        nc.vector.tensor_copy(wrep[:, 2 * D:4 * D], wrep[:, 0:2 * D])
        nc.vector.tensor_copy(wrep[:, 4 * D:6 * D], wrep[:, 2 * D:4 * D])

        # products: gpsimd multiplies the early pieces (flat 2D APs,
        # separate dst tiles - the v1 in-place 3D form ran 4.1us)
        pts = [None] * NP
        for i in GP_PIECES:
            pts[i] = rp.tile([P, LP * D], F32, tag="prod", name=f"pt{i}")
            nc.gpsimd.tensor_tensor(pts[i][:], xts[i][:], wrep[:], mult)

        # mask cast + weight mask ride gpsimd AFTER the mults (they are
        # only needed for the tail; in v2 they gated the first mult on
        # the mask DMA landing)
        mf = sp.tile([P, L], F32)
        nc.gpsimd.tensor_copy(mf[:], mi[:])
        wm = sp.tile([P, L], F32)
        nc.gpsimd.tensor_mul(wm[:], mf[:], cm_sb[:])

        # xw[p, t] = sum_d x[p, t, d] * w[d]: reduces split across DVE
        # and the otherwise-idle ACT engine (Copy activation with
        # accum_out gives one 256-wide row sum per op); DVE also
        # multiplies the late pieces.  Queues are enqueued in expected
        # data-readiness order (engines execute in-order).
        xw = sp.tile([P, L], F32)
        atrash = sp.tile([P, D], F32)

        def _reduce(i):
            p3 = pts[i][:].rearrange("p (l d) -> p l d", d=D)
            nc.vector.tensor_reduce(
                xw[:, i * LP:(i + 1) * LP], p3, axis=AX, op=add
            )

        def _act_reduce(i):
            for l in range(LP):
                col = i * LP + l
                nc.scalar.activation(
                    atrash[:], pts[i][:, l * D:(l + 1) * D], ACT.Copy,
                    accum_out=xw[:, col:col + 1],
                )

        def _vmult(i):
            pts[i] = rp.tile([P, LP * D], F32, tag="prod", name=f"pt{i}")
            nc.vector.tensor_tensor(pts[i][:], xts[i][:], wrep[:], mult)

        _reduce(0)
        _reduce(1)
        _act_reduce(2)
        _act_reduce(3)
        _vmult(5)
        _reduce(5)
        _act_reduce(4)
        _vmult(6)
        _reduce(6)
        _vmult(7)
        _reduce(7)

        # masked exp, chunk totals, cross-chunk exclusive suffix via matmul
        em = sp.tile([P, L], F32)
        nc.scalar.activation(em[:], xw[:], ACT.Exp)
        lnwarm = sp.tile([P, 1], F32)
        nc.scalar.activation(lnwarm[:], cm_sb[:, 1:2], ACT.Ln)
        nc.vector.tensor_mul(em[:], em[:], mf[:])
        tot = sp.tile([P, 1], F32)
        nc.vector.tensor_reduce(tot[:], em[:], axis=AX, op=add)
        aps = pp.tile([P, 1], F32, tag="mm")
        nc.tensor.matmul(aps[:], u_sb[:], tot[:], start=True, stop=True)
        a_sb = sp.tile([P, 1], F32)
        # + EPS seeds every suffix sum, keeping ln() finite on
        # fully-masked tails
        nc.vector.tensor_scalar_add(a_sb[:], aps[:], EPS)

        # within-chunk suffix sums, seeded with the later-chunk total
        ss = sp.tile([P, L], F32)
        nc.vector.tensor_tensor_scan(
            ss[:][:, ::-1], em[:][:, ::-1], em[:][:, ::-1],
            initial=a_sb[:], op0=add, op1=bypass,
        )
        lt = sp.tile([P, L], F32)
        nc.scalar.activation(lt[:], ss[:], ACT.Ln)

        # loss terms: sum over valid groups of (ln(suffix) - xw), and count
        diff = sp.tile([P, L], F32)
        nc.vector.tensor_sub(diff[:], lt[:], xw[:])
        res = sp.tile([P, 2], F32)
        nc.vector.scalar_tensor_tensor(
            out=diff[:], in0=diff[:], scalar=1.0, in1=wm[:],
            op0=bypass, op1=mult, accum_out=res[:, 0:1],
        )
        nc.vector.tensor_reduce(res[:, 1:2], mf[:], axis=AX, op=add)
        nc.sync.dma_start(out, res[:])

    nc.compile()
    return nc


def _host_consts():
    w_idx = np.arange(P)
    um = (
        (w_idx[:, None] // CH == w_idx[None, :] // CH)
        & (w_idx[:, None] % CH > w_idx[None, :] % CH)
    ).astype(np.float32)
    cm = np.ones((P, L), np.float32)
    cm[w_idx % CH == 0, 0] = 0.0
    return um, cm


def kernel(**inputs) -> np.ndarray:
    enc = np.ascontiguousarray(np.asarray(inputs["encoder_output"], np.float32))
    mask = np.ascontiguousarray(np.asarray(inputs["mask"], np.int32))
    w_fc = np.asarray(inputs["w_fc"], np.float32)

    if "nc" not in _cache:
        _cache["nc"] = _build_nc()
    nc = _cache["nc"]

    wt = np.ascontiguousarray(np.broadcast_to(w_fc[HID:], (P, D)), np.float32)
    um, cm = _host_consts()
    in_maps = [
        {
            "x": enc[c * BS:(c + 1) * BS],
            "mk": mask[c * BS:(c + 1) * BS],
            "wt": wt,
            "um": um,
            "cm": cm,
        }
        for c in range(NCORES)
    ]
    res = bass_utils.run_bass_kernel_spmd(
        nc, in_maps, core_ids=list(range(NCORES))
    )
    o = np.stack([r["out"] for r in res.results]).astype(np.float64)
    num = o[:, :, 0].sum()
    den = o[:, :, 1].sum() - B
    return np.asarray(num / den, dtype=np.float32)
